# revision 26
# baseline (speedup 1.0000x reference)
"""Trainium2 Bass kernel for nn_BiLinearInteractionLayer.

Math: x:(B=4096, F=32, D=64) f32, W:(P=496, D=64, D=64) f32 (torch Linear
layout: out_e = sum_d in_d * W[e, d]).  For each pair p=(i,j), i<j:
    out[b, p, e] = (sum_d x[b,i,d] * W[p,e,d]) * x[b,j,e]

Strategy (data-parallel over batch, 8 cores x 512 rows):

The harness gate is rel_err < 2e-2 (normalized by the global max), so the
whole pipeline runs in fp16: single-pass fp16 matmuls (fp32 PSUM
accumulate), fp16 elementwise multiply, fp16 output stores that the host
widens back to f32 (measured ~8e-4 rel err; fp8 weights were tried and
fail the gate at 2.7e-2).  Versus the fp32-exact baseline this halves PE
matmul passes, removes the hi/lo split entirely, and halves HBM store
traffic (the dominant cost: output is 65MB/core in f32, 32.5MB in fp16).

All matmuls are k=128 even though the contraction is only 64 deep: rows
64-127 of both operands are zeroed once at startup (bitcast-u32 memsets
spread over DVE/ACT/GPSIMD while they are otherwise idle; both sides
zero so stale-SBUF NaN/Inf can't poison 0*x).  k=64 matmuls under-report
to the HAM activity monitor and the PE never un-throttles from 1.2 GHz;
with k=128 the PE ramps to 2.4 GHz over the run (~410ns avg for a 512-col
matmul; measured identical for zero-padded and duplicated-real operands,
so the cheap zero-pad wins - it keeps loads at 8.4MB/core vs 14.4MB).

Host preformatting: x ships natively in fp16 (persistent [128, 4*2048]
tile, batch-tile column blocks) for the elementwise right-field operand;
x^T (64, bt*F*128) for the matmul stationary operand (persistent
[128,16K] tile, loaded per-bt-slice so the first matmul only waits on
0.5MB); W as wt[d, p*64+e] (64, P*64).  No on-chip transposes.

Loads are split across BOTH HWDGE rings in first-needed order (x/xt on
Sync ahead of all stores, weights on Activation with group 0 split in
half) - each dma_start costs a ~2.3us pipeline bubble on its ring, so
one ring cannot deliver the inputs before the first batch-tile needs
them (measured: 8MB of weights on one ring arrive over 37us).

Per 128-row batch tile, per left field i: pair matmuls go in bank-
aligned chunks of <= 8 pairs into 2-bank PSUM tiles (<= 16 pairs each,
bufs=4 for PE run-ahead), then a per-subfield evict / elementwise
multiply (amortizes the ~150-600ns fixed cost per instruction).  The
combine is routed per subfield across three paths, balanced at build
time with trace-fitted per-element rates (DVE-direct and GPSIMD carry
~64/36; the all-fp16 DVE 2x path is rate-dominated at this granularity):
  D: DVE tensor_mul direct from PSUM (1x mode: fp32 operand)
  A: ACT evicts PSUM->SBUF fp16, DVE tensor_mul all-fp16/SBUF (2x)
  P: ACT evicts, GPSIMD does the multiply (GPSIMD has no PSUM port)
Outputs accumulate in per-group (4 left fields) fp16 tiles and store once
per group: 8 stores/bt with 4-15KB contiguous runs per partition.

HBM traffic/core: 32.5MB out + 4MB wt + 2MB xt + 2MB x = 40.5MB.
"""
import numpy as np

import concourse.bacc as bacc
import concourse.tile as tile
import concourse.mybir as mybir
from concourse.bass_utils import run_bass_kernel_spmd

B = 4096
F = 32
D = 64
P = F * (F - 1) // 2  # 496
N_CORES = 8
BL = B // N_CORES     # 512 rows per core
BT = 128              # batch tile (SBUF partitions)
NBT = BL // BT        # 4 batch tiles per core
CHUNK = 8             # pairs per matmul chunk (8*64 = 512 = one PSUM bank)
SUBF = 32             # pairs per PSUM tile / combine instruction (4 banks)
TGROUP = 4            # left fields per output-store group
NLEFT = F - 1         # left fields 0..30

f32 = mybir.dt.float32
f16 = mybir.dt.float16
u32 = mybir.dt.uint32

_nc_cache = None


def _off(i):
    """Pair index of the first pair with left field i."""
    return 31 * i - i * (i - 1) // 2


_GROUPS = [(g0, min(TGROUP, NLEFT - g0)) for g0 in range(0, NLEFT, TGROUP)]

# trace-fitted per-instruction engine costs: ns/elem (per lane), fixed ns
_ACT_RATE, _ACT_FIX = 0.836, 260.0
_DVE1_RATE, _DVE1_FIX = 1.041, 147.0   # tensor_tensor with PSUM f32 operand
_DVE2_RATE, _DVE2_FIX = 0.632, 576.0   # tensor_tensor all-SBUF fp16 (2x)
_POOL_RATE, _POOL_FIX = 1.907, 290.0   # gpsimd tensor_tensor
_MEMSET_RATE = 0.9                     # ns per u32 elem, any engine


class _Balancer:
    """Greedy per-subfield route chooser minimizing the max engine load."""

    def __init__(self):
        self.act = 0.0
        self.dve = 0.0
        self.pool = 0.0

    def pick(self, e):
        cand = {
            "D": (0.0, _DVE1_RATE * e + _DVE1_FIX, 0.0),
            "A": (_ACT_RATE * e + _ACT_FIX, _DVE2_RATE * e + _DVE2_FIX, 0.0),
            "P": (_ACT_RATE * e + _ACT_FIX, 0.0, _POOL_RATE * e + _POOL_FIX),
        }
        best, best_load = None, None
        for r, (a, d, p) in cand.items():
            load = max(self.act + a, self.dve + d, self.pool + p)
            if best_load is None or load < best_load:
                best, best_load = r, load
        a, d, p = cand[best]
        self.act += a
        self.dve += d
        self.pool += p
        return best


def _build():
    nc = bacc.Bacc("TRN2", target_bir_lowering=False, debug=False,
                   num_devices=N_CORES)
    # x_n[r, bt*F*D + f*D + e] = x[bt*BT + r, f, e]
    x_in = nc.dram_tensor("x", [BT, NBT * F * D], f16,
                          kind="ExternalInput").ap()
    # xt[d, (bt*F + f)*BT + r] = x[bt*BT + r, f, d]
    xt_in = nc.dram_tensor("xt", [D, NBT * F * BT], f16,
                           kind="ExternalInput").ap()
    # wt[d, p*D + e] = W[p, e, d]
    wt_in = nc.dram_tensor("wt", [D, P * D], f16, kind="ExternalInput").ap()
    out = nc.dram_tensor("out", [BL, P * D], f16, kind="ExternalOutput").ap()

    bal = _Balancer()

    with tile.TileContext(nc) as tc:
        with (
            tc.tile_pool(name="consts", bufs=1) as consts,
            tc.tile_pool(name="otp", bufs=3) as otp,
            tc.tile_pool(name="pm16p", bufs=6) as pm16p,
            tc.tile_pool(name="psm", bufs=2, space="PSUM") as psm,
        ):
            # persistent inputs: transposed x, native x, weights in 4
            # merged tiles of 8 left fields each (fewer dma_starts: each
            # costs a ~2.3us pipeline bubble on its ring)
            xt_all = consts.tile([2 * D, NBT * F * BT], f16, tag="xta")
            x_all = consts.tile([BT, NBT * F * D], f16, tag="xna")
            WFIELDS = 8  # left fields per weight tile
            wt_t = []
            for wi in range(4):
                f0 = WFIELDS * wi
                f1 = min(WFIELDS * (wi + 1), NLEFT)
                c0 = _off(f0) * D
                c1 = _off(f1) * D
                t = consts.tile([2 * D, c1 - c0], f16, tag=f"wt{wi}")
                wt_t.append(t)

            # zero rows 64-127 (the fake contraction half) via u32 memsets,
            # spread over the three idle engines in first-needed order
            def _zero(sl, eng):
                if eng is nc.scalar:
                    eng.memzero(sl)
                else:
                    eng.memset(sl.bitcast(u32), 0)

            def z_xt(bt, eng):
                _zero(xt_all[D:2 * D, bt * F * BT:(bt + 1) * F * BT], eng)
                return F * BT // 2 * _MEMSET_RATE

            def z_wt(wi, eng, part=None):
                w = wt_t[wi].shape[1]
                lo, hi = 0, w
                if part is not None:
                    mid = w // 2 // 2 * 2
                    lo, hi = (0, mid) if part == 0 else (mid, w)
                _zero(wt_t[wi][D:2 * D, lo:hi], eng)
                return (hi - lo) // 2 * _MEMSET_RATE

            # first-needed first, on DVE/GPSIMD only (the ACT queue must
            # issue the weight DMAs immediately); early big tiles split by
            # column across both engines so they finish in ~3us
            bal.dve += z_xt(0, nc.vector)
            bal.pool += z_wt(0, nc.gpsimd, 0)
            bal.dve += z_wt(0, nc.vector, 1)
            bal.pool += z_wt(1, nc.gpsimd, 0)
            bal.dve += z_wt(1, nc.vector, 1)
            bal.dve += z_xt(1, nc.vector)
            bal.pool += z_wt(2, nc.gpsimd)
            bal.dve += z_wt(3, nc.vector)
            bal.pool += z_xt(2, nc.gpsimd)
            bal.dve += z_xt(3, nc.vector)

            # x/xt loads ride the Sync ring ahead of all stores; weights on
            # the Activation ring (group 0 split for the earliest matmul)
            def load_xt(bt):
                sl = slice(bt * F * BT, (bt + 1) * F * BT)
                nc.sync.dma_start(out=xt_all[0:D, sl], in_=xt_in[:, sl])

            def load_x(bt0, bt1):
                sl = slice(bt0 * F * D, bt1 * F * D)
                nc.sync.dma_start(out=x_all[:, sl], in_=x_in[:, sl])

            def load_wt(wi, half=None, eng=None):
                f0 = WFIELDS * wi
                f1 = min(WFIELDS * (wi + 1), NLEFT)
                c0 = _off(f0) * D
                c1 = _off(f1) * D
                base = c0
                if half is not None:
                    mid = (c0 + c1) // 2 // D * D
                    c0, c1 = (c0, mid) if half == 0 else (mid, c1)
                (eng or nc.scalar).dma_start(
                    out=wt_t[wi][0:D, c0 - base:c1 - base],
                    in_=wt_in[:, c0:c1])

            load_xt(0)
            load_x(0, 2)
            load_xt(1)
            load_xt(2)
            load_x(2, 4)
            load_xt(3)
            # three parallel load channels: x/xt on Sync, wt0/wt1 on the
            # Activation ring, wt2/wt3 on the GPSIMD SWDGE queue
            load_wt(0, 0)
            load_wt(0, 1)
            load_wt(1)
            load_wt(2, eng=nc.gpsimd)
            load_wt(3, eng=nc.gpsimd)
            bal.pool += 2 * 1200.0

            for bt in range(NBT):
                rows = slice(bt * BT, (bt + 1) * BT)
                xoff = bt * F * D
                for gi, (g0, gn) in enumerate(_GROUPS):
                    gbase = _off(g0) * D
                    gsz = (_off(g0 + gn) - _off(g0)) * D
                    ot = otp.tile([BT, gsz], f16, tag="ot")
                    for i in range(g0, g0 + gn):
                        npair = F - 1 - i  # pairs (i, i+1..31), consecutive
                        p0 = _off(i)
                        wt = wt_t[i // WFIELDS]
                        wbase = _off(i // WFIELDS * WFIELDS) * D
                        lhsT = xt_all[:, (bt * F + i) * BT:
                                      (bt * F + i + 1) * BT]  # [128, 128]
                        for s0 in range(0, npair, SUBF):
                            sn = min(SUBF, npair - s0)
                            n_s = sn * D
                            # one 4-bank PSUM tile per whole field
                            pm = psm.tile([BT, 4 * CHUNK * D], f32, tag="mm")
                            for c0 in range(0, sn, CHUNK):
                                cn = min(CHUNK, sn - c0)
                                n = cn * D
                                ws = (p0 + s0 + c0) * D - wbase
                                nc.tensor.matmul(
                                    pm[:, c0 * D:c0 * D + n], lhsT,
                                    wt[:, ws:ws + n],
                                    start=True, stop=True)
                            j0 = xoff + (i + 1 + s0) * D
                            xj = x_all[:, j0:j0 + n_s]
                            osl = (p0 + s0) * D - gbase
                            ot_sl = ot[:, osl:osl + n_s]
                            route = bal.pick(n_s)
                            if route == "D":
                                nc.vector.tensor_mul(ot_sl, pm[:, :n_s], xj)
                            else:
                                pm16 = pm16p.tile([BT, n_s], f16, tag="pm16")
                                nc.scalar.copy(pm16, pm[:, :n_s])
                                if route == "A":
                                    nc.vector.tensor_mul(ot_sl, pm16, xj)
                                else:
                                    nc.gpsimd.tensor_mul(ot_sl, pm16, xj)
                    nc.sync.dma_start(out=out[rows, gbase:gbase + gsz],
                                      in_=ot)
    nc.compile()
    return nc


def _get_nc():
    global _nc_cache
    if _nc_cache is None:
        _nc_cache = _build()
    return _nc_cache


def _prep_inputs(x, W):
    x16 = np.asarray(x, dtype=np.float16).reshape(N_CORES, BL, F * D)
    # native x regrouped so batch-tile row r is the partition: [BT, NBT*F*D]
    xn = np.ascontiguousarray(
        x16.reshape(N_CORES, NBT, BT, F * D).transpose(0, 2, 1, 3)
    ).reshape(N_CORES, BT, NBT * F * D)
    # per-core pre-transposed layout: [D, NBT, F, BT] flattened
    xt = np.ascontiguousarray(
        x16.reshape(N_CORES, NBT, BT, F, D).transpose(0, 4, 1, 3, 2)
    ).reshape(N_CORES, D, NBT * F * BT)
    wt = np.ascontiguousarray(
        np.asarray(W, dtype=np.float32).transpose(2, 0, 1)
    ).reshape(D, P * D).astype(np.float16)
    return xn, xt, wt


def _run(x, W, trace=False, trace_kwargs=None):
    xn, xt, wt = _prep_inputs(x, W)
    in_maps = [{"x": xn[c], "xt": xt[c], "wt": wt}
               for c in range(N_CORES)]
    res = run_bass_kernel_spmd(_get_nc(), in_maps, list(range(N_CORES)),
                               trace=trace, **(trace_kwargs or {}))
    outs = [res.results[c]["out"].reshape(BL, P, D) for c in range(N_CORES)]
    return np.concatenate(outs, axis=0).astype(np.float32), res


def kernel(x, W):
    out, _ = _run(x, W)
    return out


# revision 28
# speedup vs baseline: 1.1316x; 1.1316x over previous
"""Trainium2 Bass kernel for nn_BiLinearInteractionLayer.

Math: x:(B=4096, F=32, D=64) f32, W:(P=496, D=64, D=64) f32 (torch Linear
layout: out_e = sum_d in_d * W[e, d]).  For each pair p=(i,j), i<j:
    out[b, p, e] = (sum_d x[b,i,d] * W[p,e,d]) * x[b,j,e]

Strategy (data-parallel over batch, 8 cores x 512 rows):

The harness gate is rel_err < 2e-2 (normalized by the global max), so the
whole pipeline runs in fp16: single-pass fp16 matmuls (fp32 PSUM
accumulate), fp16 elementwise multiply, fp16 output stores that the host
widens back to f32 (measured ~8e-4 rel err; fp8 weights were tried and
fail the gate at 2.7e-2).  Versus the fp32-exact baseline this halves PE
matmul passes, removes the hi/lo split entirely, and halves HBM store
traffic (the dominant cost: output is 65MB/core in f32, 32.5MB in fp16).

All matmuls are k=128 even though the contraction is only 64 deep: rows
64-127 of both operands are zeroed once at startup (bitcast-u32 memsets
spread over DVE/ACT/GPSIMD while they are otherwise idle; both sides
zero so stale-SBUF NaN/Inf can't poison 0*x).  k=64 matmuls under-report
to the HAM activity monitor and the PE never un-throttles from 1.2 GHz;
with k=128 the PE ramps to 2.4 GHz over the run (~410ns avg for a 512-col
matmul; measured identical for zero-padded and duplicated-real operands,
so the cheap zero-pad wins - it keeps loads at 8.4MB/core vs 14.4MB).

Host preformatting: x ships natively in fp16 (persistent [128, 4*2048]
tile, batch-tile column blocks) for the elementwise right-field operand;
x^T (64, bt*F*128) for the matmul stationary operand (persistent
[128,16K] tile, loaded per-bt-slice so the first matmul only waits on
0.5MB); W as wt[d, p*64+e] (64, P*64).  No on-chip transposes.

Loads are split across BOTH HWDGE rings in first-needed order (x/xt on
Sync ahead of all stores, weights on Activation with group 0 split in
half) - each dma_start costs a ~2.3us pipeline bubble on its ring, so
one ring cannot deliver the inputs before the first batch-tile needs
them (measured: 8MB of weights on one ring arrive over 37us).

Per 128-row batch tile, per left field i: pair matmuls go in bank-
aligned chunks of <= 8 pairs into 2-bank PSUM tiles (<= 16 pairs each,
bufs=4 for PE run-ahead), then a per-subfield evict / elementwise
multiply (amortizes the ~150-600ns fixed cost per instruction).  The
combine is routed per subfield across three paths, balanced at build
time with trace-fitted per-element rates (DVE-direct and GPSIMD carry
~64/36; the all-fp16 DVE 2x path is rate-dominated at this granularity):
  D: DVE tensor_mul direct from PSUM (1x mode: fp32 operand)
  A: ACT evicts PSUM->SBUF fp16, DVE tensor_mul all-fp16/SBUF (2x)
  P: ACT evicts, GPSIMD does the multiply (GPSIMD has no PSUM port)
Outputs accumulate in per-group (4 left fields) fp16 tiles and store once
per group: 8 stores/bt with 4-15KB contiguous runs per partition.

HBM traffic/core: 32.5MB out + 4MB wt + 2MB xt + 2MB x = 40.5MB.
"""
import numpy as np

import concourse.bacc as bacc
import concourse.tile as tile
import concourse.mybir as mybir
from concourse.bass_utils import run_bass_kernel_spmd

B = 4096
F = 32
D = 64
P = F * (F - 1) // 2  # 496
N_CORES = 8
BL = B // N_CORES     # 512 rows per core
BT = 128              # batch tile (SBUF partitions)
NBT = BL // BT        # 4 batch tiles per core
CHUNK = 8             # pairs per matmul chunk (8*64 = 512 = one PSUM bank)
SUBF = 16             # pairs per PSUM tile / combine instruction (2 banks)
TGROUP = 8            # left fields per output-store group / weight tile
NLEFT = F - 1         # left fields 0..30

f32 = mybir.dt.float32
f16 = mybir.dt.float16
u32 = mybir.dt.uint32

_nc_cache = None


def _off(i):
    """Pair index of the first pair with left field i."""
    return 31 * i - i * (i - 1) // 2


_GROUPS = [(g0, min(TGROUP, NLEFT - g0)) for g0 in range(0, NLEFT, TGROUP)]

# trace-fitted per-instruction engine costs: ns/elem (per lane), fixed ns
_ACT_RATE, _ACT_FIX = 0.836, 260.0
_DVE1_RATE, _DVE1_FIX = 1.041, 147.0   # tensor_tensor with PSUM f32 operand
_DVE2_RATE, _DVE2_FIX = 0.632, 576.0   # tensor_tensor all-SBUF fp16 (2x)
_POOL_RATE, _POOL_FIX = 1.907, 290.0   # gpsimd tensor_tensor
_MEMSET_RATE = 0.9                     # ns per u32 elem, any engine


class _Balancer:
    """Greedy per-subfield route chooser minimizing the max engine load."""

    def __init__(self):
        self.act = 0.0
        self.dve = 0.0
        self.pool = 0.0

    def pick(self, e):
        cand = {
            "D": (0.0, _DVE1_RATE * e + _DVE1_FIX, 0.0),
            "A": (_ACT_RATE * e + _ACT_FIX, _DVE2_RATE * e + _DVE2_FIX, 0.0),
            "P": (_ACT_RATE * e + _ACT_FIX, 0.0, _POOL_RATE * e + _POOL_FIX),
        }
        best, best_load = None, None
        for r, (a, d, p) in cand.items():
            load = max(self.act + a, self.dve + d, self.pool + p)
            if best_load is None or load < best_load:
                best, best_load = r, load
        a, d, p = cand[best]
        self.act += a
        self.dve += d
        self.pool += p
        return best


def _build():
    nc = bacc.Bacc("TRN2", target_bir_lowering=False, debug=False,
                   num_devices=N_CORES)
    # x_n[r, bt*F*D + f*D + e] = x[bt*BT + r, f, e]
    x_in = nc.dram_tensor("x", [BT, NBT * F * D], f16,
                          kind="ExternalInput").ap()
    # xt[d, (bt*F + f)*BT + r] = x[bt*BT + r, f, d]
    xt_in = nc.dram_tensor("xt", [D, NBT * F * BT], f16,
                           kind="ExternalInput").ap()
    # wt[d, p*D + e] = W[p, e, d]
    wt_in = nc.dram_tensor("wt", [D, P * D], f16, kind="ExternalInput").ap()
    out = nc.dram_tensor("out", [BL, P * D], f16, kind="ExternalOutput").ap()

    bal = _Balancer()

    with tile.TileContext(nc) as tc:
        with (
            tc.tile_pool(name="consts", bufs=1) as consts,
            tc.tile_pool(name="otp", bufs=2) as otp,
            tc.tile_pool(name="pm16p", bufs=6) as pm16p,
            tc.tile_pool(name="psm", bufs=4, space="PSUM") as psm,
        ):
            # persistent inputs: transposed x, native x, per-group weights
            xt_all = consts.tile([2 * D, NBT * F * BT], f16, tag="xta")
            x_all = consts.tile([BT, NBT * F * D], f16, tag="xna")
            wt_g = []
            for gi, (g0, gn) in enumerate(_GROUPS):
                c0 = _off(g0) * D
                c1 = _off(g0 + gn) * D
                t = consts.tile([2 * D, c1 - c0], f16, tag=f"wt{gi}")
                wt_g.append(t)

            # zero rows 64-127 (the fake contraction half) via u32 memsets,
            # spread over the three idle engines in first-needed order
            def _zero(sl, eng):
                if eng is nc.scalar:
                    eng.memzero(sl)
                else:
                    eng.memset(sl.bitcast(u32), 0)

            def z_xt(bt, eng):
                _zero(xt_all[D:2 * D, bt * F * BT:(bt + 1) * F * BT], eng)
                return F * BT // 2 * _MEMSET_RATE

            def z_wt(gi, eng, part=None):
                w = wt_g[gi].shape[1]
                lo, hi = 0, w
                if part is not None:
                    mid = w // 2 // 2 * 2
                    lo, hi = (0, mid) if part == 0 else (mid, w)
                _zero(wt_g[gi][D:2 * D, lo:hi], eng)
                return (hi - lo) // 2 * _MEMSET_RATE

            # first-needed first, on DVE/GPSIMD only (the ACT queue must
            # issue the weight DMAs immediately); the early big tiles are
            # column-split across both engines so they finish in ~3us
            bal.dve += z_xt(0, nc.vector)
            bal.pool += z_wt(0, nc.gpsimd, 0)
            bal.dve += z_wt(0, nc.vector, 1)
            bal.pool += z_wt(1, nc.gpsimd, 0)
            bal.dve += z_wt(1, nc.vector, 1)
            bal.dve += z_xt(1, nc.vector)
            bal.pool += z_wt(2, nc.gpsimd)
            bal.dve += z_wt(3, nc.vector)
            bal.pool += z_xt(2, nc.gpsimd)
            bal.dve += z_xt(3, nc.vector)

            # x/xt loads ride the Sync ring ahead of all stores; weights on
            # the Activation ring (group 0 split for the earliest matmul)
            def load_xt(bt):
                sl = slice(bt * F * BT, (bt + 1) * F * BT)
                nc.sync.dma_start(out=xt_all[0:D, sl], in_=xt_in[:, sl])

            def load_x(bt0, bt1):
                sl = slice(bt0 * F * D, bt1 * F * D)
                nc.sync.dma_start(out=x_all[:, sl], in_=x_in[:, sl])

            def load_wt(gi, half=None, eng=None):
                c0 = _off(_GROUPS[gi][0]) * D
                c1 = _off(_GROUPS[gi][0] + _GROUPS[gi][1]) * D
                if half is not None:
                    mid = (c0 + c1) // 2 // D * D
                    c0, c1 = (c0, mid) if half == 0 else (mid, c1)
                base = _off(_GROUPS[gi][0]) * D
                (eng or nc.scalar).dma_start(
                    out=wt_g[gi][0:D, c0 - base:c1 - base],
                    in_=wt_in[:, c0:c1])

            # weight tiles 0-1 stream on the Activation ring; 2-3 slot
            # into the Sync ring's idle gaps between the x/xt loads
            load_xt(0)
            load_x(0, 2)
            load_wt(2, eng=nc.sync)
            load_xt(1)
            load_xt(2)
            load_wt(3, eng=nc.sync)
            load_x(2, 4)
            load_xt(3)
            load_wt(0, 0)
            load_wt(0, 1)
            load_wt(1, 0)
            load_wt(1, 1)

            for bt in range(NBT):
                rows = slice(bt * BT, (bt + 1) * BT)
                xoff = bt * F * D
                for gi, (g0, gn) in enumerate(_GROUPS):
                    gbase = _off(g0) * D
                    gsz = (_off(g0 + gn) - _off(g0)) * D
                    ot = otp.tile([BT, gsz], f16, tag="ot")
                    for i in range(g0, g0 + gn):
                        npair = F - 1 - i  # pairs (i, i+1..31), consecutive
                        p0 = _off(i)
                        lhsT = xt_all[:, (bt * F + i) * BT:
                                      (bt * F + i + 1) * BT]  # [128, 128]
                        for s0 in range(0, npair, SUBF):
                            sn = min(SUBF, npair - s0)
                            n_s = sn * D
                            # one 2-bank PSUM tile per <=16-pair subfield
                            pm = psm.tile([BT, 2 * CHUNK * D], f32, tag="mm")
                            for c0 in range(0, sn, CHUNK):
                                cn = min(CHUNK, sn - c0)
                                n = cn * D
                                ws = (p0 + s0 + c0) * D - gbase
                                nc.tensor.matmul(
                                    pm[:, c0 * D:c0 * D + n], lhsT,
                                    wt_g[gi][:, ws:ws + n],
                                    start=True, stop=True)
                            j0 = xoff + (i + 1 + s0) * D
                            xj = x_all[:, j0:j0 + n_s]
                            osl = (p0 + s0) * D - gbase
                            ot_sl = ot[:, osl:osl + n_s]
                            route = bal.pick(n_s)
                            if route == "D":
                                nc.vector.tensor_mul(ot_sl, pm[:, :n_s], xj)
                            else:
                                pm16 = pm16p.tile([BT, n_s], f16, tag="pm16")
                                nc.scalar.copy(pm16, pm[:, :n_s])
                                if route == "A":
                                    nc.vector.tensor_mul(ot_sl, pm16, xj)
                                else:
                                    nc.gpsimd.tensor_mul(ot_sl, pm16, xj)
                    nc.sync.dma_start(out=out[rows, gbase:gbase + gsz],
                                      in_=ot)
    nc.compile()
    return nc


def _get_nc():
    global _nc_cache
    if _nc_cache is None:
        _nc_cache = _build()
    return _nc_cache


def _prep_inputs(x, W):
    x16 = np.asarray(x, dtype=np.float16).reshape(N_CORES, BL, F * D)
    # native x regrouped so batch-tile row r is the partition: [BT, NBT*F*D]
    xn = np.ascontiguousarray(
        x16.reshape(N_CORES, NBT, BT, F * D).transpose(0, 2, 1, 3)
    ).reshape(N_CORES, BT, NBT * F * D)
    # per-core pre-transposed layout: [D, NBT, F, BT] flattened
    xt = np.ascontiguousarray(
        x16.reshape(N_CORES, NBT, BT, F, D).transpose(0, 4, 1, 3, 2)
    ).reshape(N_CORES, D, NBT * F * BT)
    wt = np.ascontiguousarray(
        np.asarray(W, dtype=np.float32).transpose(2, 0, 1)
    ).reshape(D, P * D).astype(np.float16)
    return xn, xt, wt


def _run(x, W, trace=False, trace_kwargs=None):
    xn, xt, wt = _prep_inputs(x, W)
    in_maps = [{"x": xn[c], "xt": xt[c], "wt": wt}
               for c in range(N_CORES)]
    res = run_bass_kernel_spmd(_get_nc(), in_maps, list(range(N_CORES)),
                               trace=trace, **(trace_kwargs or {}))
    outs = [res.results[c]["out"].reshape(BL, P, D) for c in range(N_CORES)]
    return np.concatenate(outs, axis=0).astype(np.float32), res


def kernel(x, W):
    out, _ = _run(x, W)
    return out


# revision 29
# speedup vs baseline: 1.2309x; 1.0877x over previous
"""Trainium2 Bass kernel for nn_BiLinearInteractionLayer.

Math: x:(B=4096, F=32, D=64) f32, W:(P=496, D=64, D=64) f32 (torch Linear
layout: out_e = sum_d in_d * W[e, d]).  For each pair p=(i,j), i<j:
    out[b, p, e] = (sum_d x[b,i,d] * W[p,e,d]) * x[b,j,e]

Strategy (data-parallel over batch, 8 cores x 512 rows):

The harness gate is rel_err < 2e-2 (normalized by the global max), so the
whole pipeline runs in fp16: single-pass fp16 matmuls (fp32 PSUM
accumulate), fp16 elementwise multiply, fp16 output stores that the host
widens back to f32 (measured ~8e-4 rel err; fp8 weights were tried and
fail the gate at 2.7e-2).  Versus the fp32-exact baseline this halves PE
matmul passes, removes the hi/lo split entirely, and halves HBM store
traffic (the dominant cost: output is 65MB/core in f32, 32.5MB in fp16).

All matmuls are k=128 even though the contraction is only 64 deep: rows
64-127 of both operands are zeroed once at startup (bitcast-u32 memsets
spread over DVE/ACT/GPSIMD while they are otherwise idle; both sides
zero so stale-SBUF NaN/Inf can't poison 0*x).  k=64 matmuls under-report
to the HAM activity monitor and the PE never un-throttles from 1.2 GHz;
with k=128 the PE ramps to 2.4 GHz over the run (~410ns avg for a 512-col
matmul; measured identical for zero-padded and duplicated-real operands,
so the cheap zero-pad wins - it keeps loads at 8.4MB/core vs 14.4MB).

Host preformatting: x ships natively in fp16 (persistent [128, 4*2048]
tile, batch-tile column blocks) for the elementwise right-field operand;
x^T (64, bt*F*128) for the matmul stationary operand (persistent
[128,16K] tile, loaded per-bt-slice so the first matmul only waits on
0.5MB); W as wt[d, p*64+e] (64, P*64).  No on-chip transposes.

Loads are split across BOTH HWDGE rings in first-needed order (x/xt on
Sync ahead of all stores, weights on Activation with group 0 split in
half) - each dma_start costs a ~2.3us pipeline bubble on its ring, so
one ring cannot deliver the inputs before the first batch-tile needs
them (measured: 8MB of weights on one ring arrive over 37us).

Per 128-row batch tile, per left field i: pair matmuls go in bank-
aligned chunks of <= 8 pairs into 2-bank PSUM tiles (<= 16 pairs each,
bufs=4 for PE run-ahead), then a per-subfield evict / elementwise
multiply (amortizes the ~150-600ns fixed cost per instruction).  The
combine is routed per subfield across three paths, balanced at build
time with trace-fitted per-element rates (DVE-direct and GPSIMD carry
~64/36; the all-fp16 DVE 2x path is rate-dominated at this granularity):
  D: DVE tensor_mul direct from PSUM (1x mode: fp32 operand)
  A: ACT evicts PSUM->SBUF fp16, DVE tensor_mul all-fp16/SBUF (2x)
  P: ACT evicts, GPSIMD does the multiply (GPSIMD has no PSUM port)
Outputs accumulate in per-group (4 left fields) fp16 tiles and store once
per group: 8 stores/bt with 4-15KB contiguous runs per partition.

HBM traffic/core: 32.5MB out + 4MB wt + 2MB xt + 2MB x = 40.5MB.
"""
import numpy as np

import concourse.bacc as bacc
import concourse.tile as tile
import concourse.mybir as mybir
from concourse.bass_utils import run_bass_kernel_spmd

B = 4096
F = 32
D = 64
P = F * (F - 1) // 2  # 496
N_CORES = 8
BL = B // N_CORES     # 512 rows per core
BT = 128              # batch tile (SBUF partitions)
NBT = BL // BT        # 4 batch tiles per core
CHUNK = 8             # pairs per matmul chunk (8*64 = 512 = one PSUM bank)
SUBF = 16             # pairs per PSUM tile / combine instruction (2 banks)
TGROUP = 4            # left fields per output-store group
NLEFT = F - 1         # left fields 0..30

f32 = mybir.dt.float32
f16 = mybir.dt.float16
u32 = mybir.dt.uint32

_nc_cache = None


def _off(i):
    """Pair index of the first pair with left field i."""
    return 31 * i - i * (i - 1) // 2


_GROUPS = [(g0, min(TGROUP, NLEFT - g0)) for g0 in range(0, NLEFT, TGROUP)]

# trace-fitted per-instruction engine costs: ns/elem (per lane), fixed ns
_ACT_RATE, _ACT_FIX = 0.836, 260.0
_DVE1_RATE, _DVE1_FIX = 1.041, 147.0   # tensor_tensor with PSUM f32 operand
_DVE2_RATE, _DVE2_FIX = 0.632, 576.0   # tensor_tensor all-SBUF fp16 (2x)
_POOL_RATE, _POOL_FIX = 1.907, 290.0   # gpsimd tensor_tensor
_MEMSET_RATE = 0.9                     # ns per u32 elem, any engine


class _Balancer:
    """Greedy per-subfield route chooser minimizing the max engine load."""

    def __init__(self):
        self.act = 0.0
        self.dve = 0.0
        self.pool = 0.0

    def pick(self, e):
        cand = {
            "D": (0.0, _DVE1_RATE * e + _DVE1_FIX, 0.0),
            "A": (_ACT_RATE * e + _ACT_FIX, _DVE2_RATE * e + _DVE2_FIX, 0.0),
            "P": (_ACT_RATE * e + _ACT_FIX, 0.0, _POOL_RATE * e + _POOL_FIX),
        }
        best, best_load = None, None
        for r, (a, d, p) in cand.items():
            load = max(self.act + a, self.dve + d, self.pool + p)
            if best_load is None or load < best_load:
                best, best_load = r, load
        a, d, p = cand[best]
        self.act += a
        self.dve += d
        self.pool += p
        return best


def _build():
    nc = bacc.Bacc("TRN2", target_bir_lowering=False, debug=False,
                   num_devices=N_CORES)
    # x_n[r, bt*F*D + f*D + e] = x[bt*BT + r, f, e]
    x_in = nc.dram_tensor("x", [BT, NBT * F * D], f16,
                          kind="ExternalInput").ap()
    # xt[d, (bt*F + f)*BT + r] = x[bt*BT + r, f, d]
    xt_in = nc.dram_tensor("xt", [D, NBT * F * BT], f16,
                           kind="ExternalInput").ap()
    # wt[d, p*D + e] = W[p, e, d]
    wt_in = nc.dram_tensor("wt", [D, P * D], f16, kind="ExternalInput").ap()
    out = nc.dram_tensor("out", [BL, P * D], f16, kind="ExternalOutput").ap()

    bal = _Balancer()

    with tile.TileContext(nc) as tc:
        with (
            tc.tile_pool(name="consts", bufs=1) as consts,
            tc.tile_pool(name="otp", bufs=4) as otp,
            tc.tile_pool(name="pm16p", bufs=8) as pm16p,
            tc.tile_pool(name="psm", bufs=4, space="PSUM") as psm,
        ):
            # persistent inputs: transposed x, native x, per-group weights
            xt_all = consts.tile([2 * D, NBT * F * BT], f16, tag="xta")
            x_all = consts.tile([BT, NBT * F * D], f16, tag="xna")
            wt_g = []
            for gi, (g0, gn) in enumerate(_GROUPS):
                c0 = _off(g0) * D
                c1 = _off(g0 + gn) * D
                t = consts.tile([2 * D, c1 - c0], f16, tag=f"wt{gi}")
                wt_g.append(t)

            # zero rows 64-127 (the fake contraction half) via u32 memsets,
            # spread over the three idle engines in first-needed order
            def _zero(sl, eng):
                if eng is nc.scalar:
                    eng.memzero(sl)
                else:
                    eng.memset(sl.bitcast(u32), 0)

            def z_xt(bt, eng):
                _zero(xt_all[D:2 * D, bt * F * BT:(bt + 1) * F * BT], eng)
                return F * BT // 2 * _MEMSET_RATE

            def z_wt(gi, eng):
                _zero(wt_g[gi][D:2 * D, :], eng)
                return (wt_g[gi].shape[1] // 2) * _MEMSET_RATE

            bal.dve += z_xt(0, nc.vector)
            bal.dve += z_wt(0, nc.vector)
            bal.pool += z_wt(1, nc.gpsimd)
            bal.act += z_wt(2, nc.scalar)
            bal.dve += z_xt(1, nc.vector)
            bal.pool += z_wt(3, nc.gpsimd)
            bal.act += z_wt(4, nc.scalar)
            bal.dve += z_wt(5, nc.vector)
            bal.pool += z_xt(2, nc.gpsimd)
            bal.act += z_wt(6, nc.scalar)
            bal.dve += z_wt(7, nc.vector)
            bal.pool += z_xt(3, nc.gpsimd)

            # x/xt loads ride the Sync ring ahead of all stores; weights on
            # the Activation ring (group 0 split for the earliest matmul)
            def load_xt(bt):
                sl = slice(bt * F * BT, (bt + 1) * F * BT)
                nc.sync.dma_start(out=xt_all[0:D, sl], in_=xt_in[:, sl])

            def load_x(bt0, bt1):
                sl = slice(bt0 * F * D, bt1 * F * D)
                nc.sync.dma_start(out=x_all[:, sl], in_=x_in[:, sl])

            def load_wt(gi, half=None):
                c0 = _off(_GROUPS[gi][0]) * D
                c1 = _off(_GROUPS[gi][0] + _GROUPS[gi][1]) * D
                if half is not None:
                    mid = (c0 + c1) // 2 // D * D
                    c0, c1 = (c0, mid) if half == 0 else (mid, c1)
                base = _off(_GROUPS[gi][0]) * D
                nc.scalar.dma_start(out=wt_g[gi][0:D, c0 - base:c1 - base],
                                    in_=wt_in[:, c0:c1])

            load_xt(0)
            load_x(0, 2)
            load_xt(1)
            load_xt(2)
            load_x(2, 4)
            load_xt(3)
            load_wt(0, 0)
            load_wt(0, 1)
            for gi in range(1, len(_GROUPS)):
                load_wt(gi)

            for bt in range(NBT):
                rows = slice(bt * BT, (bt + 1) * BT)
                xoff = bt * F * D
                for gi, (g0, gn) in enumerate(_GROUPS):
                    gbase = _off(g0) * D
                    gsz = (_off(g0 + gn) - _off(g0)) * D
                    ot = otp.tile([BT, gsz], f16, tag="ot")
                    for i in range(g0, g0 + gn):
                        npair = F - 1 - i  # pairs (i, i+1..31), consecutive
                        p0 = _off(i)
                        lhsT = xt_all[:, (bt * F + i) * BT:
                                      (bt * F + i + 1) * BT]  # [128, 128]
                        for s0 in range(0, npair, SUBF):
                            sn = min(SUBF, npair - s0)
                            n_s = sn * D
                            # one 2-bank PSUM tile per <=16-pair subfield
                            pm = psm.tile([BT, 2 * CHUNK * D], f32, tag="mm")
                            for c0 in range(0, sn, CHUNK):
                                cn = min(CHUNK, sn - c0)
                                n = cn * D
                                ws = (p0 + s0 + c0) * D - gbase
                                nc.tensor.matmul(
                                    pm[:, c0 * D:c0 * D + n], lhsT,
                                    wt_g[gi][:, ws:ws + n],
                                    start=True, stop=True)
                            j0 = xoff + (i + 1 + s0) * D
                            xj = x_all[:, j0:j0 + n_s]
                            osl = (p0 + s0) * D - gbase
                            ot_sl = ot[:, osl:osl + n_s]
                            route = bal.pick(n_s)
                            if route == "D":
                                nc.vector.tensor_mul(ot_sl, pm[:, :n_s], xj)
                            else:
                                pm16 = pm16p.tile([BT, n_s], f16, tag="pm16")
                                nc.scalar.copy(pm16, pm[:, :n_s])
                                if route == "A":
                                    nc.vector.tensor_mul(ot_sl, pm16, xj)
                                else:
                                    nc.gpsimd.tensor_mul(ot_sl, pm16, xj)
                    nc.sync.dma_start(out=out[rows, gbase:gbase + gsz],
                                      in_=ot)
    nc.compile()
    return nc


def _get_nc():
    global _nc_cache
    if _nc_cache is None:
        _nc_cache = _build()
    return _nc_cache


def _prep_inputs(x, W):
    x16 = np.asarray(x, dtype=np.float16).reshape(N_CORES, BL, F * D)
    # native x regrouped so batch-tile row r is the partition: [BT, NBT*F*D]
    xn = np.ascontiguousarray(
        x16.reshape(N_CORES, NBT, BT, F * D).transpose(0, 2, 1, 3)
    ).reshape(N_CORES, BT, NBT * F * D)
    # per-core pre-transposed layout: [D, NBT, F, BT] flattened
    xt = np.ascontiguousarray(
        x16.reshape(N_CORES, NBT, BT, F, D).transpose(0, 4, 1, 3, 2)
    ).reshape(N_CORES, D, NBT * F * BT)
    wt = np.ascontiguousarray(
        np.asarray(W, dtype=np.float32).transpose(2, 0, 1)
    ).reshape(D, P * D).astype(np.float16)
    return xn, xt, wt


def _run(x, W, trace=False, trace_kwargs=None):
    xn, xt, wt = _prep_inputs(x, W)
    in_maps = [{"x": xn[c], "xt": xt[c], "wt": wt}
               for c in range(N_CORES)]
    res = run_bass_kernel_spmd(_get_nc(), in_maps, list(range(N_CORES)),
                               trace=trace, **(trace_kwargs or {}))
    outs = [res.results[c]["out"].reshape(BL, P, D) for c in range(N_CORES)]
    return np.concatenate(outs, axis=0).astype(np.float32), res


def kernel(x, W):
    out, _ = _run(x, W)
    return out
